# revision 52
# baseline (speedup 1.0000x reference)
# GAT (2-layer, DGL GATConv mean-path) on 8 TRN2 NeuronCores.
#
# Sharding: nodes partitioned by dst across 8 cores (2500 each); edges live
# on the core owning their dst node. Per core:
#  - node pass (replicated): ft1 = x @ [W1 d-major | el | er] in bf16; rows
#    [ft 256 d-major | el 8 | er 8] written compact (272 cols) into 768B-
#    stride table1 rows; pad cols pre-zeroed by a stride-0 DRAM->DRAM
#    broadcast. Writes issue on the Pool queue, copies split DVE/Act.
#  - layer-1 edge pass: dst-sorted edges in 128-edge chunks, 16 chunks per
#    dma_gather. Gathers move 768B rows as f32 (bitcast) to halve the
#    element count; compute reads bf16 bitcast views. er[dst] is distributed
#    to edge rows by per-chunk PE matmuls with host-built one-hot S_ve (fp8)
#    as lhsT (no dst-side gather); er2[dst] likewise via deferred per-block
#    matmuls. ft stored d-major so the ft*exp(lrelu(el+er)) multiply
#    broadcasts on a middle dim -> DVE 2x mode. Scatter-by-dst via PE
#    matmuls with a host-built resident one-hot slab (fp8) accumulating
#    [num 256 | den 8] per dst block in PSUM.
#  - layer-2 exchange: compact [2560 x 21] rows ([ft2 40 fp8 | el2 bf16]
#    bit-packed) in a single AllGather placed after the L1 Pool stream,
#    then a cheap strided D2D relayout into 256B-stride table2 rows.
#  - layer-2 edge pass: same machinery, one scalar head; out = num/den.
# exp() without max-subtraction is exact for the softmax ratio (values are
# O(1); reference subtracts the segment max only for overflow safety).
#
# Perf (CoreSim cost model, 8 cores): 524us (prev) -> ~233us.
import math
import sys
from contextlib import ExitStack

import numpy as np
import ml_dtypes

if "/opt/trn_rl_repo" not in sys.path:
    sys.path.insert(0, "/opt/trn_rl_repo")

import concourse.bass as bass
import concourse.mybir as mybir
import concourse.tile as tile
from concourse import bacc
from concourse.masks import make_identity

F32 = mybir.dt.float32
BF16 = mybir.dt.bfloat16
FP8 = mybir.dt.float8e4
I16 = mybir.dt.int16
I64 = mybir.dt.int64
AX = mybir.AxisListType
ALU = mybir.AluOpType
ACTF = mybir.ActivationFunctionType

NP_BF16 = ml_dtypes.bfloat16
NP_FP8 = ml_dtypes.float8_e4m3


class Cfg:
    def __init__(self, N=20000, F=128, H1=8, D1=32, C=40, ncores=8, neg=0.2):
        self.N, self.F, self.H1, self.D1, self.C = N, F, H1, D1, C
        self.NCORES, self.NEG = ncores, neg
        self.FT1 = H1 * D1                       # 256
        self.NB = N // ncores                    # 2500
        self.BLK = 128
        self.NBLK = math.ceil(self.NB / self.BLK)    # 20
        self.NPAD = self.NBLK * self.BLK             # 2560
        self.CH = 128
        self.GB = 16
        self.NIDX = self.GB * self.CH                # 2048 idx / gather
        self.W1X = self.FT1 + 2 * H1                 # 272 rhs1 cols
        self.STEP1 = 384                             # table1 row stride (768B)
        self.EL1 = self.FT1                          # el at cols 256..264
        self.ER1 = self.FT1 + H1                     # er at cols 264..272
        self.C2 = C + 2                              # [ft2 40 | el2 | er2]
        self.T2W = 21          # t2own row: [ft2 40 fp8 | el2 bf16] = 42 B
        self.STEP2 = 128                             # table2 row stride (256B)
        self.GB2 = 16                                # L2 chunks per gather
        self.NIDX2 = self.GB2 * self.CH
        self.NT = math.ceil(N / 128)                 # 157 node tiles
        self.NSUB = 16
        self.KT2 = self.FT1 // 128                   # 2
        # single collective: cheapest total Pool-chain time (collectives
        # serialize the Pool instruction stream, so grouping only stalls L1)
        self.GRPS = [20]
        assert sum(self.GRPS) == self.NBLK


CFG = Cfg()


# ---------------------------------------------------------------- host prep

def _wrap16(a):
    """Edge-index array [E_pad] -> dma_gather idx layout [128, E_pad//16]."""
    cols = np.ascontiguousarray(np.asarray(a).reshape(-1, 16).T).astype(np.int16)
    return np.ascontiguousarray(np.tile(cols, (8, 1)))


def _host_prep(cfg, src, dst):
    """Sort/bucket edges by dst; same chunk structure on all cores."""
    src = np.asarray(src).astype(np.int64)
    dst = np.asarray(dst).astype(np.int64)
    perm = np.argsort(dst, kind="stable")
    ss, dd = src[perm], dst[perm]
    core_of = dd // cfg.NB
    rem = dd % cfg.NB
    blk_of = rem // cfg.BLK
    vloc_of = rem % cfg.BLK

    counts = np.zeros((cfg.NCORES, cfg.NBLK), np.int64)
    np.add.at(counts, (core_of, blk_of), 1)
    chunks_pb = np.maximum(1, -(-counts // cfg.CH)).max(axis=0)     # [NBLK]
    K = int(chunks_pb.sum())
    K1 = -(-K // cfg.GB) * cfg.GB            # layer-1 loop extent
    K_pad = -(-K1 // cfg.GB2) * cfg.GB2      # table extent (layer-2 loop)
    cb = np.concatenate(
        [np.repeat(np.arange(cfg.NBLK), chunks_pb),
         np.full(K_pad - K, cfg.NBLK - 1)]).astype(np.int64)
    blk_start_chunk = np.concatenate([[0], np.cumsum(chunks_pb)])
    E_pad = K_pad * cfg.CH

    # group / table2 row mapping
    grp_of_blk = np.repeat(np.arange(len(cfg.GRPS)), cfg.GRPS)
    grp_start_blk = np.concatenate([[0], np.cumsum(cfg.GRPS)])[:-1]
    grp_rows = np.array(cfg.GRPS) * cfg.BLK * cfg.NCORES
    grp_base = np.concatenate([[0], np.cumsum(grp_rows)])[:-1]

    grps_arr = np.asarray(cfg.GRPS)

    def row2_of(n):
        n = np.asarray(n)
        c = n // cfg.NB
        loc = n % cfg.NB
        b = loc // cfg.BLK
        g = grp_of_blk[b]
        return (grp_base[g] + c * grps_arr[g] * cfg.BLK
                + (b - grp_start_blk[g]) * cfg.BLK + loc % cfg.BLK)

    src_e = np.zeros((cfg.NCORES, E_pad), np.int64)
    vloc_e = np.full((cfg.NCORES, E_pad), 999, np.int64)
    for c in range(cfg.NCORES):
        mc = core_of == c
        sc, bc, vc = ss[mc], blk_of[mc], vloc_of[mc]
        for b in range(cfg.NBLK):
            mb = bc == b
            cnt = int(mb.sum())
            off = int(blk_start_chunk[b]) * cfg.CH
            src_e[c, off:off + cnt] = sc[mb]
            vloc_e[c, off:off + cnt] = vc[mb]

    # one-hot tables (fp8): st[e, k*128+v] (scatter lhsT) and
    # sve[v, k*128+e] (er-distribute lhsT)
    st_l, sve_l, si1_l, si2_l = [], [], [], []
    for c in range(cfg.NCORES):
        v = vloc_e[c].reshape(K_pad, cfg.CH)            # [k, e]
        st = np.zeros((128, K_pad, 128), np.float32)
        sve = np.zeros((128, K_pad, 128), np.float32)
        kk, ee = np.nonzero(v < 128)
        vv = v[kk, ee]
        st[ee, kk, vv] = 1.0
        sve[vv, kk, ee] = 1.0
        st_l.append(np.ascontiguousarray(
            st.reshape(128, K_pad * 128).astype(NP_FP8)))
        sve_l.append(np.ascontiguousarray(
            sve.reshape(128, K_pad * 128).astype(NP_FP8)))
        si1_l.append(_wrap16(src_e[c]))
        si2_l.append(_wrap16(row2_of(src_e[c])))

    # chunk lists per block (for deferred er2 matmuls; exclude global padding
    # chunks — their sve columns are zero and sb_er2e is pre-zeroed)
    blk_chunks = [list(range(int(blk_start_chunk[b]),
                             int(blk_start_chunk[b + 1])))
                  for b in range(cfg.NBLK)]

    return dict(K_pad=K_pad, K1=K1, cb=cb, st=st_l, sve=sve_l, si1=si1_l, si2=si2_l,
                blk_chunks=blk_chunks, grp_of_blk=grp_of_blk,
                grp_start_blk=grp_start_blk, grp_base=grp_base)


def _dmajor_perm(H, D):
    """ft column permutation: new col d*H+h = old col h*D+d."""
    p = np.arange(H * D).reshape(H, D).T.reshape(-1)   # p[d*H+h] = h*D+d
    return p


# ---------------------------------------------------------------- bass build

def _bcast(ap, axis, n):
    shape = list(ap.shape)
    shape.insert(axis, n)
    return ap.unsqueeze(axis).broadcast_to(shape)


def _build(cfg, prep):
    nc = bacc.Bacc(None, target_bir_lowering=False, num_devices=cfg.NCORES)
    N, F, H1, D1, C = cfg.N, cfg.F, cfg.H1, cfg.D1, cfg.C
    FT1, GB, CH, NBLK, BLK = cfg.FT1, cfg.GB, cfg.CH, cfg.NBLK, cfg.BLK
    K_pad, K1, cb = prep["K_pad"], prep["K1"], prep["cb"]
    NT1 = N + 256                        # table1 rows incl. own-slab overrun

    def din(name, shape, dt=F32):
        return nc.declare_dram_parameter(name, list(shape), dt, isOutput=False)

    xT = din("xT", [F, N], BF16)
    rhs1_in = din("rhs1", [F, cfg.W1X], BF16)          # [ft dmaj|el|er] cols
    rhs2_in = din("rhs2", [128, cfg.KT2 * cfg.C2], BF16)
    st_in = din("st", [128, K_pad * 128], FP8)
    sve_in = din("sve", [128, K_pad * 128], FP8)
    si1 = din("si1", [128, K_pad * 8], I16)
    si2 = din("si2", [128, K_pad * 8], I16)
    out = nc.declare_dram_parameter("out", [cfg.NPAD, C], F32, isOutput=True)

    table1 = nc.dram_tensor("table1", [NT1, cfg.STEP1], BF16)
    zrow = nc.dram_tensor("zrow", [128, 16], BF16)
    t2own = nc.dram_tensor("t2own", [cfg.NPAD, cfg.T2W], BF16)
    table2c = nc.dram_tensor(
        "table2c", [NBLK * cfg.NCORES * BLK, cfg.T2W], BF16,
        addr_space="Shared")
    table2 = nc.dram_tensor("table2", [NBLK * cfg.NCORES * BLK, cfg.STEP2],
                            BF16)

    first_of = [k == 0 or cb[k - 1] != cb[k] for k in range(K_pad)]
    last_of1 = [k == K1 - 1 or cb[k + 1] != cb[k] for k in range(K1)]
    last_of2 = [k == K_pad - 1 or cb[k + 1] != cb[k] for k in range(K_pad)]
    grp_of_blk = prep["grp_of_blk"]
    grp_start_blk = prep["grp_start_blk"]
    grp_base = prep["grp_base"]

    with tile.TileContext(nc) as tc:
        with ExitStack() as ctx:
            nidx_reg = nc.gpsimd.to_reg(cfg.NIDX)
            consts = ctx.enter_context(tc.tile_pool(name="consts", bufs=1))

            sb_rhs1 = consts.tile([128, cfg.W1X], BF16)
            sb_rhs2 = consts.tile([128, cfg.KT2, cfg.C2], BF16)
            sb_si1 = consts.tile([128, K_pad * 8], I16)
            sb_si2 = consts.tile([128, K_pad * 8], I16)
            # scatter one-hot slab, resident for both layers (halves loaded
            # on separate queues)
            sb_st = consts.tile([128, K_pad, 128], FP8)
            kh = K_pad // 2
            nc.sync.dma_start(
                out=sb_st[:, 0:kh, :],
                in_=st_in[:, 0:kh * 128].rearrange("p (k e) -> p k e", k=kh))
            nc.scalar.dma_start(
                out=sb_st[:, kh:K_pad, :],
                in_=st_in[:, kh * 128:].rearrange(
                    "p (k e) -> p k e", k=K_pad - kh))
            sb_ident = consts.tile([128, 128], BF16)
            sb_elown = consts.tile([128, NBLK, H1], BF16)
            sb_erown = consts.tile([128, NBLK, H1], BF16)
            sb_er2own = consts.tile([128, NBLK, 1], BF16)
            sb_er2e = consts.tile([128, K_pad], BF16)
            nc.vector.memset(sb_er2e, 0.0)

            nc.sync.dma_start(out=sb_rhs1, in_=rhs1_in[:, :])
            nc.sync.dma_start(
                out=sb_rhs2,
                in_=rhs2_in[:, :].rearrange("p (t c) -> p t c", t=cfg.KT2))
            nc.sync.dma_start(out=sb_si1, in_=si1[:, :])
            nc.sync.dma_start(out=sb_si2, in_=si2[:, :])
            make_identity(nc, sb_ident)

            # ---- stage 1: node pass -> table1 (strided 272-col writes) ----
            NSUB = cfg.NSUB
            n_super = math.ceil(cfg.NT / NSUB)
            with tc.tile_pool(name="ps_node", bufs=8, space="PSUM") as ps_node, \
                 tc.tile_pool(name="xt", bufs=3) as xtp, \
                 tc.tile_pool(name="row1", bufs=3) as rowp:
                for s in range(n_super):
                    t0 = s * NSUB * 128
                    ncols = min(NSUB * 128, N - t0)
                    nsub = math.ceil(ncols / 128)
                    xt_t = xtp.tile([128, NSUB * 128], BF16)
                    nc.sync.dma_start(out=xt_t[:, :ncols],
                                      in_=xT[:, t0:t0 + ncols])
                    # pad cols 272..384 of table1 are zero-filled by the
                    # stride-0 D2D broadcast; rows here are compact 272
                    row_b = rowp.tile([128, NSUB, cfg.W1X], BF16)
                    for i in range(nsub):
                        nt = min(128, ncols - i * 128)
                        ps = ps_node.tile([128, cfg.W1X], F32)
                        nc.tensor.matmul(ps[:nt, :],
                                         xt_t[:, i * 128:i * 128 + nt],
                                         sb_rhs1, start=True, stop=True)
                        if i % 2 == 0:
                            nc.vector.tensor_copy(row_b[:nt, i, :], ps[:nt, :])
                        else:
                            nc.scalar.copy(row_b[:nt, i, :], ps[:nt, :])
                    full = ncols // 128
                    if full:
                        dst_ap = bass.AP(
                            tensor=table1, offset=t0 * cfg.STEP1,
                            ap=[[cfg.STEP1, 128], [128 * cfg.STEP1, full],
                                [1, cfg.W1X]])
                        nc.gpsimd.dma_start(
                            out=dst_ap, in_=row_b[:, :full, :])
                    if full < nsub:
                        nt = ncols - full * 128
                        dst_ap = bass.AP(
                            tensor=table1, offset=(t0 + full * 128) * cfg.STEP1,
                            ap=[[cfg.STEP1, nt], [1, cfg.W1X]])
                        nc.gpsimd.dma_start(
                            out=dst_ap, in_=row_b[:nt, full, :])

            # zero-fill the table1 tail (rows N..NT1) so the own-slab gather
            # of the last core never reads uninitialized DRAM
            ztail = consts.tile([128, 2, cfg.STEP1], BF16)
            nc.vector.memset(ztail, 0.0)
            tail_ap = bass.AP(
                tensor=table1, offset=N * cfg.STEP1,
                ap=[[cfg.STEP1, 128], [128 * cfg.STEP1, 2], [1, cfg.STEP1]])
            nc.sync.dma_start(out=tail_ap, in_=ztail)

            # zero DRAM row: source for stride-0 broadcast zero-fills
            zrow_t = consts.tile([128, 16], BF16)
            nc.vector.memset(zrow_t, 0.0)
            nc.sync.dma_start(out=zrow[:, :], in_=zrow_t)
            # table1 pad cols (272..384) zero-filled via one D2D broadcast
    # (in first dim stride 0 re-reads the same zero row)
            half1 = NT1 // 2
            for off, cnt in ((0, half1), (half1, NT1 - half1)):
                pad_in = bass.AP(tensor=zrow, offset=0,
                                 ap=[[0, cnt], [1, cfg.STEP1 - cfg.W1X]])
                pad_out = bass.AP(tensor=table1,
                                  offset=off * cfg.STEP1 + cfg.W1X,
                                  ap=[[cfg.STEP1, cnt],
                                      [1, cfg.STEP1 - cfg.W1X]])
                nc.sync.dma_start(out=pad_out, in_=pad_in)

            # own-node el/er slab via a per-core row gather (indices
            # core*NB + i, supplied per core so the SPMD program is shared)
            ownidx = din("ownidx", [128, cfg.NPAD // 16], I16)
            sb_ownidx = consts.tile([128, cfg.NPAD // 16], I16)
            nc.sync.dma_start(out=sb_ownidx, in_=ownidx[:, :])
            nown_reg = nc.gpsimd.to_reg(cfg.NPAD)
            with tc.tile_pool(name="own", bufs=1) as own_p:
                own4 = own_p.tile([128, cfg.NPAD // 128, 64], F32)
                nc.gpsimd.dma_gather(
                    own4, table1[:, cfg.EL1:cfg.EL1 + 128].bitcast(F32),
                    sb_ownidx, cfg.NPAD, nown_reg, 64,
                    elem_step=cfg.STEP1 // 2, single_packet=False)
                own_t = own4.bitcast(BF16)
                nc.vector.tensor_copy(sb_elown, own_t[:, :, 0:H1])
                nc.vector.tensor_copy(sb_erown, own_t[:, :, H1:2 * H1])

            # ---- edge pass pools ---------------------------------------
            gt_p = ctx.enter_context(tc.tile_pool(name="gt", bufs=4))
            # sve tiles are referenced by deferred er2 matmuls up to ~4
            # batches back (last block + padding) — keep 5 bufs
            sve_p = ctx.enter_context(tc.tile_pool(name="sveb", bufs=5))
            e_p = ctx.enter_context(tc.tile_pool(name="eb", bufs=2))
            w_p = ctx.enter_context(tc.tile_pool(name="wb", bufs=3))
            fin_p = ctx.enter_context(tc.tile_pool(name="fin", bufs=2))

            # ---- stage 2: layer-1 edge pass ----------------------------
            l1ctx = ExitStack()
            ps_o = l1ctx.enter_context(tc.tile_pool(name="ps_o", bufs=2, space="PSUM"))
            ps_er = l1ctx.enter_context(tc.tile_pool(name="ps_er", bufs=2, space="PSUM"))
            ps_er2 = l1ctx.enter_context(tc.tile_pool(name="ps_er2", bufs=1, space="PSUM"))
            ps_t = l1ctx.enter_context(tc.tile_pool(name="ps_t", bufs=1, space="PSUM"))
            ps_b = l1ctx.enter_context(tc.tile_pool(name="ps_b", bufs=2, space="PSUM"))
            sve_tiles = {}
            gt = st_t = w_b = ex_b = None
            ps_acc = None
            for k in range(K1):
                g, c = divmod(k, GB)
                if c == 0:
                    gt4 = gt_p.tile([128, GB, cfg.STEP1 // 2], F32)
                    nc.gpsimd.dma_gather(
                        gt4, table1[:, :].bitcast(F32),
                        sb_si1[:, g * 128:(g + 1) * 128],
                        cfg.NIDX, nidx_reg, cfg.STEP1 // 2,
                        single_packet=False)
                    gt = gt4.bitcast(BF16)
                    sve_t = sve_p.tile([128, GB, 128], FP8, tag="sveb")
                    nc.sync.dma_start(
                        out=sve_t, in_=sve_in[:, g * 2048:(g + 1) * 2048]
                        .rearrange("p (g e) -> p g e", g=GB))
                    sve_tiles[g] = sve_t
                    # er[dst] per edge via one matmul per chunk
                    er_ps = ps_er.tile([128, GB, H1], F32)
                    for cc in range(GB):
                        bb = int(cb[g * GB + cc])
                        nc.tensor.matmul(er_ps[:, cc, :], sve_t[:, cc, :],
                                         sb_erown[:, bb, :],
                                         start=True, stop=True)
                    # e = lrelu(el + er);  ex = exp(e)
                    e_b = e_p.tile([128, GB, H1], F32, tag="e1")
                    nc.vector.tensor_add(
                        e_b, gt[:, :, cfg.EL1:cfg.EL1 + H1], er_ps)
                    e_l = e_p.tile([128, GB, H1], F32, tag="e1")
                    nc.vector.scalar_tensor_tensor(
                        e_l, e_b, cfg.NEG, e_b, ALU.mult, ALU.max)
                    ex_b = e_p.tile([128, GB, H1], BF16, tag="e1")
                    nc.scalar.activation(ex_b, e_l, ACTF.Exp)
                    # w rows: [ft*ex (d-major) | ex]
                    w_b = w_p.tile([128, GB, FT1 + H1], BF16, tag="wb")
                    nc.vector.tensor_mul(
                        w_b[:, :, 0:FT1].rearrange("p g (d h) -> p g d h", d=D1),
                        gt[:, :, 0:FT1].rearrange("p g (d h) -> p g d h", d=D1),
                        _bcast(ex_b, 2, D1))
                    nc.scalar.copy(w_b[:, :, FT1:FT1 + H1], ex_b)

                if k == K1 - 1:
                    # pre-zero table2 cols (relayout writes 21 of 128; the
                    # layer-2 gather reads whole 256B rows)
                    t2rows = NBLK * cfg.NCORES * BLK
                    for zoff in (0, t2rows // 2):
                        z_in = bass.AP(tensor=zrow, offset=0,
                                       ap=[[0, t2rows // 2], [1, cfg.STEP2]])
                        z_out = bass.AP(tensor=table2,
                                        offset=zoff * cfg.STEP2,
                                        ap=[[cfg.STEP2, t2rows // 2],
                                            [1, cfg.STEP2]])
                        nc.gpsimd.dma_start(out=z_out, in_=z_in)

                b = int(cb[k])
                if first_of[k]:
                    ps_acc = ps_o.tile([128, FT1 + H1], F32)
                nc.tensor.matmul(ps_acc, sb_st[:, k, :], w_b[:, c, :],
                                 start=first_of[k], stop=last_of1[k])
                if last_of1[k]:
                    # h = relu(num / max(den, eps))
                    s_t = fin_p.tile([128, H1], F32, tag="fin")
                    nc.vector.tensor_scalar_max(
                        s_t, ps_acc[:, FT1:FT1 + H1], 1e-16)
                    rs = fin_p.tile([128, H1], F32, tag="fin")
                    nc.vector.reciprocal(rs, s_t)
                    ht = fin_p.tile([128, FT1], F32, tag="fin")
                    nc.vector.tensor_mul(
                        ht.rearrange("p (d h) -> p d h", d=D1),
                        ps_acc[:, 0:FT1].rearrange("p (d h) -> p d h", d=D1),
                        _bcast(rs, 1, D1))
                    sb_h = fin_p.tile([128, FT1], BF16, tag="hblk")
                    nc.scalar.activation(sb_h, ht, ACTF.Relu)

                    # layer-2 node prep for block b
                    sb_hT = fin_p.tile([128, cfg.KT2, 128], BF16, tag="hTblk")
                    for half in range(cfg.KT2):
                        pst = ps_t.tile([128, 128], BF16)
                        nc.tensor.transpose(
                            pst, sb_h[:, half * 128:(half + 1) * 128],
                            sb_ident)
                        nc.scalar.copy(sb_hT[:, half, :], pst)
                    ps2 = ps_b.tile([128, cfg.C2], F32)
                    for half in range(cfg.KT2):
                        nc.tensor.matmul(
                            ps2, sb_hT[:, half, :], sb_rhs2[:, half, :],
                            start=(half == 0), stop=(half == cfg.KT2 - 1))
                    row2 = fin_p.tile([128, cfg.T2W], BF16, tag="fin2")
                    nc.scalar.copy(row2[:, 0:C // 2].bitcast(FP8), ps2[:, 0:C])
                    nc.scalar.copy(row2[:, C // 2:C // 2 + 1], ps2[:, C:C + 1])
                    nc.scalar.copy(sb_er2own[:, b, :], ps2[:, C + 1:C + 2])
                    nc.sync.dma_start(
                        out=t2own[b * BLK:(b + 1) * BLK, 0:cfg.T2W], in_=row2)

                    # deferred er2 distribution for block b's chunks
                    ch_l = prep["blk_chunks"][b]
                    er2_ps = ps_er2.tile([128, max(16, len(ch_l))], F32)
                    for j, kk in enumerate(ch_l):
                        gg, cc = divmod(kk, GB)
                        nc.tensor.matmul(
                            er2_ps[:, j:j + 1], sve_tiles[gg][:, cc, :],
                            sb_er2own[:, b, :], start=True, stop=True)
                    k0 = ch_l[0]
                    nc.vector.tensor_copy(
                        sb_er2e[:, k0:k0 + len(ch_l)],
                        er2_ps[:, 0:len(ch_l)])

                    # grouped allgather once a group's blocks are all done
                    gidx = int(grp_of_blk[b])
                    if b == int(grp_start_blk[gidx]) + cfg.GRPS[gidx] - 1:
                        b0 = int(grp_start_blk[gidx])
                        nblks = cfg.GRPS[gidx]
                        rows = nblks * BLK
                        base = int(grp_base[gidx])
                        nc.gpsimd.collective_compute(
                            "AllGather", ALU.bypass,
                            replica_groups=[list(range(cfg.NCORES))],
                            ins=[t2own[b0 * BLK:b0 * BLK + rows, :].opt()],
                            outs=[table2c[base:base + cfg.NCORES * rows, :].opt()])
                        # relayout compact 42-byte rows -> 256B-stride rows
                        # (NEFF requires contiguous collective outputs)
                        nrows = cfg.NCORES * rows
                        src_ap = bass.AP(
                            tensor=table2c, offset=base * cfg.T2W,
                            ap=[[cfg.T2W, nrows], [1, cfg.T2W]])
                        dst_ap = bass.AP(
                            tensor=table2, offset=base * cfg.STEP2,
                            ap=[[cfg.STEP2, nrows], [1, cfg.T2W]])
                        nc.sync.dma_start(out=dst_ap, in_=src_ap)

            # ---- stage 3: layer-2 edge pass ----------------------------
            l1ctx.close()
            ps_o2 = ctx.enter_context(
                tc.tile_pool(name="ps_o2", bufs=2, space="PSUM"))
            GB2 = cfg.GB2
            nidx2_reg = nc.gpsimd.to_reg(cfg.NIDX2)
            g2 = w2_b = None
            ps_acc2 = None
            HB = 16                              # compute half-batch
            for k in range(K_pad):
                g, c = divmod(k, GB2)
                if c == 0:
                    g24 = gt_p.tile([128, GB2, cfg.STEP2 // 2], F32, tag="gt")
                    nc.gpsimd.dma_gather(
                        g24, table2[:, :].bitcast(F32),
                        sb_si2[:, g * 128:(g + 1) * 128],
                        cfg.NIDX2, nidx2_reg, cfg.STEP2 // 2,
                        single_packet=False)
                    g2 = g24.bitcast(BF16)
                    w2_b = w_p.tile([128, GB2, C + 1], BF16, tag="wb2")
                if c % HB == 0:
                    h0 = c
                    g2h = g2[:, h0:h0 + HB, :]
                    e2 = e_p.tile([128, HB, 1], F32, tag="e2")
                    nc.vector.tensor_add(
                        e2, g2h[:, :, C // 2:C // 2 + 1],
                        _bcast(sb_er2e[:, g * GB2 + h0:g * GB2 + h0 + HB], 2, 1))
                    e2l = e_p.tile([128, HB, 1], F32, tag="e2")
                    nc.vector.scalar_tensor_tensor(
                        e2l, e2, cfg.NEG, e2, ALU.mult, ALU.max)
                    ex2 = e_p.tile([128, HB, 1], BF16, tag="e2")
                    nc.scalar.activation(ex2, e2l, ACTF.Exp)
                    meng = nc.vector if (g % 2 == 0) else nc.gpsimd
                    meng.tensor_mul(
                        w2_b[:, h0:h0 + HB, 0:C],
                        g2h[:, :, 0:C // 2].bitcast(FP8),
                        _bcast(ex2.squeeze(2), 2, C))
                    nc.vector.tensor_copy(
                        w2_b[:, h0:h0 + HB, C:C + 1], ex2)

                b = int(cb[k])
                if first_of[k]:
                    ps_acc2 = ps_o2.tile([128, C + 1], F32)
                nc.tensor.matmul(ps_acc2, sb_st[:, k, :], w2_b[:, c, :],
                                 start=first_of[k], stop=last_of2[k])
                if last_of2[k]:
                    s2 = fin_p.tile([128, 1], F32, tag="finL2")
                    nc.vector.tensor_scalar_max(s2, ps_acc2[:, C:C + 1], 1e-16)
                    rs2 = fin_p.tile([128, 1], F32, tag="finL2")
                    nc.vector.reciprocal(rs2, s2)
                    ot = fin_p.tile([128, C], F32, tag="finL2")
                    nc.vector.tensor_mul(ot, ps_acc2[:, 0:C],
                                         _bcast(rs2.squeeze(1), 1, C))
                    nc.sync.dma_start(out=out[b * BLK:(b + 1) * BLK, :], in_=ot)

    nc.finalize()
    return nc


# ---------------------------------------------------------------- driver

def _make_in_maps(cfg, inputs, prep):
    x = np.asarray(inputs["x"], np.float32)
    W1 = np.asarray(inputs["W1"], np.float32)
    al1 = np.asarray(inputs["attn_l1"], np.float32)
    ar1 = np.asarray(inputs["attn_r1"], np.float32)
    W2 = np.asarray(inputs["W2"], np.float32)
    al2 = np.asarray(inputs["attn_l2"], np.float32).reshape(-1)
    ar2 = np.asarray(inputs["attn_r2"], np.float32).reshape(-1)

    pm = _dmajor_perm(cfg.H1, cfg.D1)
    W1p = W1[:, pm]                                    # d-major ft cols
    el_cols = np.einsum("fhd,hd->fh", W1.reshape(cfg.F, cfg.H1, cfg.D1), al1)
    er_cols = np.einsum("fhd,hd->fh", W1.reshape(cfg.F, cfg.H1, cfg.D1), ar1)
    rhs1 = np.concatenate([W1p, el_cols, er_cols], axis=1)

    W2p = W2[pm, :]                                    # K rows d-major
    el2_col = W2 @ al2                                 # [256]
    er2_col = W2 @ ar2
    rhs2 = np.concatenate(
        [W2p, el2_col[pm, None], er2_col[pm, None]], axis=1)  # [256, 42]
    rhs2 = np.ascontiguousarray(
        rhs2.reshape(cfg.KT2, 128, cfg.C2).transpose(1, 0, 2).reshape(
            128, cfg.KT2 * cfg.C2))

    common = {
        "xT": np.ascontiguousarray(x.T).astype(NP_BF16),
        "rhs1": np.ascontiguousarray(rhs1).astype(NP_BF16),
        "rhs2": rhs2.astype(NP_BF16),
    }
    in_maps = []
    for c in range(cfg.NCORES):
        m = dict(common)
        m["st"] = prep["st"][c]
        m["sve"] = prep["sve"][c]
        m["si1"] = prep["si1"][c]
        m["si2"] = prep["si2"][c]
        m["ownidx"] = _wrap16(np.arange(c * cfg.NB, c * cfg.NB + cfg.NPAD))
        in_maps.append(m)
    return in_maps


def build_all(inputs, cfg=CFG):
    prep = _host_prep(cfg, inputs["src"], inputs["dst"])
    nc = _build(cfg, prep)
    in_maps = _make_in_maps(cfg, inputs, prep)
    return nc, in_maps


def kernel(**inputs):
    cfg = CFG
    nc, in_maps = build_all(inputs, cfg)
    from concourse.bass_utils import run_bass_kernel_spmd
    res = run_bass_kernel_spmd(nc, in_maps, core_ids=list(range(cfg.NCORES)))
    out = np.concatenate(
        [res.results[c]["out"][:cfg.NB] for c in range(cfg.NCORES)], axis=0)
    return np.ascontiguousarray(out, dtype=np.float32)


# revision 53
# speedup vs baseline: 1.0068x; 1.0068x over previous
# GAT (2-layer, DGL GATConv mean-path) on 8 TRN2 NeuronCores.
#
# Sharding: nodes partitioned by dst across 8 cores (2500 each); edges live
# on the core owning their dst node. Per core:
#  - node pass (replicated): ft1 = x @ [W1 d-major | el | er] in bf16; rows
#    [ft 256 d-major | el 8 | er 8] written compact (272 cols) into 768B-
#    stride table1 rows; pad cols pre-zeroed by a stride-0 DRAM->DRAM
#    broadcast. Writes issue on the Pool queue, copies split DVE/Act.
#  - layer-1 edge pass: dst-sorted edges in 128-edge chunks, 16 chunks per
#    dma_gather. Gathers move 768B rows as f32 (bitcast) to halve the
#    element count; compute reads bf16 bitcast views. er[dst] is distributed
#    to edge rows by per-chunk PE matmuls with host-built one-hot S_ve (fp8)
#    as lhsT (no dst-side gather); er2[dst] likewise via deferred per-block
#    matmuls. ft stored d-major so the ft*exp(lrelu(el+er)) multiply
#    broadcasts on a middle dim -> DVE 2x mode. Scatter-by-dst via PE
#    matmuls with a host-built resident one-hot slab (fp8) accumulating
#    [num 256 | den 8] per dst block in PSUM.
#  - layer-2 exchange: compact [2560 x 21] rows ([ft2 40 fp8 | el2 bf16]
#    bit-packed) in a single AllGather placed after the L1 Pool stream,
#    then a cheap strided D2D relayout into 256B-stride table2 rows.
#  - layer-2 edge pass: same machinery, one scalar head; out = num/den.
# exp() without max-subtraction is exact for the softmax ratio (values are
# O(1); reference subtracts the segment max only for overflow safety).
#
# Perf (CoreSim cost model, 8 cores): 524us (prev) -> ~233us.
import math
import sys
from contextlib import ExitStack

import numpy as np
import ml_dtypes

if "/opt/trn_rl_repo" not in sys.path:
    sys.path.insert(0, "/opt/trn_rl_repo")

import concourse.bass as bass
import concourse.mybir as mybir
import concourse.tile as tile
from concourse import bacc
from concourse.masks import make_identity

F32 = mybir.dt.float32
BF16 = mybir.dt.bfloat16
FP8 = mybir.dt.float8e4
I16 = mybir.dt.int16
I64 = mybir.dt.int64
AX = mybir.AxisListType
ALU = mybir.AluOpType
ACTF = mybir.ActivationFunctionType

NP_BF16 = ml_dtypes.bfloat16
NP_FP8 = ml_dtypes.float8_e4m3


class Cfg:
    def __init__(self, N=20000, F=128, H1=8, D1=32, C=40, ncores=8, neg=0.2):
        self.N, self.F, self.H1, self.D1, self.C = N, F, H1, D1, C
        self.NCORES, self.NEG = ncores, neg
        self.FT1 = H1 * D1                       # 256
        self.NB = N // ncores                    # 2500
        self.BLK = 128
        self.NBLK = math.ceil(self.NB / self.BLK)    # 20
        self.NPAD = self.NBLK * self.BLK             # 2560
        self.CH = 128
        self.GB = 16
        self.NIDX = self.GB * self.CH                # 2048 idx / gather
        self.W1X = self.FT1 + 2 * H1                 # 272 rhs1 cols
        self.STEP1 = 384                             # table1 row stride (768B)
        self.EL1 = self.FT1                          # el at cols 256..264
        self.ER1 = self.FT1 + H1                     # er at cols 264..272
        self.C2 = C + 2                              # [ft2 40 | el2 | er2]
        self.T2W = 21          # t2own row: [ft2 40 fp8 | el2 bf16] = 42 B
        self.STEP2 = 128                             # table2 row stride (256B)
        self.GB2 = 16                                # L2 chunks per gather
        self.NIDX2 = self.GB2 * self.CH
        self.NT = math.ceil(N / 128)                 # 157 node tiles
        self.NSUB = 8
        self.KT2 = self.FT1 // 128                   # 2
        # single collective: cheapest total Pool-chain time (collectives
        # serialize the Pool instruction stream, so grouping only stalls L1)
        self.GRPS = [20]
        assert sum(self.GRPS) == self.NBLK


CFG = Cfg()


# ---------------------------------------------------------------- host prep

def _wrap16(a):
    """Edge-index array [E_pad] -> dma_gather idx layout [128, E_pad//16]."""
    cols = np.ascontiguousarray(np.asarray(a).reshape(-1, 16).T).astype(np.int16)
    return np.ascontiguousarray(np.tile(cols, (8, 1)))


def _host_prep(cfg, src, dst):
    """Sort/bucket edges by dst; same chunk structure on all cores."""
    src = np.asarray(src).astype(np.int64)
    dst = np.asarray(dst).astype(np.int64)
    perm = np.argsort(dst, kind="stable")
    ss, dd = src[perm], dst[perm]
    core_of = dd // cfg.NB
    rem = dd % cfg.NB
    blk_of = rem // cfg.BLK
    vloc_of = rem % cfg.BLK

    counts = np.zeros((cfg.NCORES, cfg.NBLK), np.int64)
    np.add.at(counts, (core_of, blk_of), 1)
    chunks_pb = np.maximum(1, -(-counts // cfg.CH)).max(axis=0)     # [NBLK]
    K = int(chunks_pb.sum())
    K1 = -(-K // cfg.GB) * cfg.GB            # layer-1 loop extent
    K_pad = -(-K1 // cfg.GB2) * cfg.GB2      # table extent (layer-2 loop)
    cb = np.concatenate(
        [np.repeat(np.arange(cfg.NBLK), chunks_pb),
         np.full(K_pad - K, cfg.NBLK - 1)]).astype(np.int64)
    blk_start_chunk = np.concatenate([[0], np.cumsum(chunks_pb)])
    E_pad = K_pad * cfg.CH

    # group / table2 row mapping
    grp_of_blk = np.repeat(np.arange(len(cfg.GRPS)), cfg.GRPS)
    grp_start_blk = np.concatenate([[0], np.cumsum(cfg.GRPS)])[:-1]
    grp_rows = np.array(cfg.GRPS) * cfg.BLK * cfg.NCORES
    grp_base = np.concatenate([[0], np.cumsum(grp_rows)])[:-1]

    grps_arr = np.asarray(cfg.GRPS)

    def row2_of(n):
        n = np.asarray(n)
        c = n // cfg.NB
        loc = n % cfg.NB
        b = loc // cfg.BLK
        g = grp_of_blk[b]
        return (grp_base[g] + c * grps_arr[g] * cfg.BLK
                + (b - grp_start_blk[g]) * cfg.BLK + loc % cfg.BLK)

    src_e = np.zeros((cfg.NCORES, E_pad), np.int64)
    vloc_e = np.full((cfg.NCORES, E_pad), 999, np.int64)
    for c in range(cfg.NCORES):
        mc = core_of == c
        sc, bc, vc = ss[mc], blk_of[mc], vloc_of[mc]
        for b in range(cfg.NBLK):
            mb = bc == b
            cnt = int(mb.sum())
            off = int(blk_start_chunk[b]) * cfg.CH
            src_e[c, off:off + cnt] = sc[mb]
            vloc_e[c, off:off + cnt] = vc[mb]

    # one-hot tables (fp8): st[e, k*128+v] (scatter lhsT) and
    # sve[v, k*128+e] (er-distribute lhsT)
    st_l, sve_l, si1_l, si2_l = [], [], [], []
    for c in range(cfg.NCORES):
        v = vloc_e[c].reshape(K_pad, cfg.CH)            # [k, e]
        st = np.zeros((128, K_pad, 128), np.float32)
        sve = np.zeros((128, K_pad, 128), np.float32)
        kk, ee = np.nonzero(v < 128)
        vv = v[kk, ee]
        st[ee, kk, vv] = 1.0
        sve[vv, kk, ee] = 1.0
        st_l.append(np.ascontiguousarray(
            st.reshape(128, K_pad * 128).astype(NP_FP8)))
        sve_l.append(np.ascontiguousarray(
            sve.reshape(128, K_pad * 128).astype(NP_FP8)))
        si1_l.append(_wrap16(src_e[c]))
        si2_l.append(_wrap16(row2_of(src_e[c])))

    # chunk lists per block (for deferred er2 matmuls; exclude global padding
    # chunks — their sve columns are zero and sb_er2e is pre-zeroed)
    blk_chunks = [list(range(int(blk_start_chunk[b]),
                             int(blk_start_chunk[b + 1])))
                  for b in range(cfg.NBLK)]

    return dict(K_pad=K_pad, K1=K1, cb=cb, st=st_l, sve=sve_l, si1=si1_l, si2=si2_l,
                blk_chunks=blk_chunks, grp_of_blk=grp_of_blk,
                grp_start_blk=grp_start_blk, grp_base=grp_base)


def _dmajor_perm(H, D):
    """ft column permutation: new col d*H+h = old col h*D+d."""
    p = np.arange(H * D).reshape(H, D).T.reshape(-1)   # p[d*H+h] = h*D+d
    return p


# ---------------------------------------------------------------- bass build

def _bcast(ap, axis, n):
    shape = list(ap.shape)
    shape.insert(axis, n)
    return ap.unsqueeze(axis).broadcast_to(shape)


def _build(cfg, prep):
    nc = bacc.Bacc(None, target_bir_lowering=False, num_devices=cfg.NCORES)
    N, F, H1, D1, C = cfg.N, cfg.F, cfg.H1, cfg.D1, cfg.C
    FT1, GB, CH, NBLK, BLK = cfg.FT1, cfg.GB, cfg.CH, cfg.NBLK, cfg.BLK
    K_pad, K1, cb = prep["K_pad"], prep["K1"], prep["cb"]
    NT1 = N + 256                        # table1 rows incl. own-slab overrun

    def din(name, shape, dt=F32):
        return nc.declare_dram_parameter(name, list(shape), dt, isOutput=False)

    xT = din("xT", [F, N], BF16)
    rhs1_in = din("rhs1", [F, cfg.W1X], BF16)          # [ft dmaj|el|er] cols
    rhs2_in = din("rhs2", [128, cfg.KT2 * cfg.C2], BF16)
    st_in = din("st", [128, K_pad * 128], FP8)
    sve_in = din("sve", [128, K_pad * 128], FP8)
    si1 = din("si1", [128, K_pad * 8], I16)
    si2 = din("si2", [128, K_pad * 8], I16)
    out = nc.declare_dram_parameter("out", [cfg.NPAD, C], F32, isOutput=True)

    table1 = nc.dram_tensor("table1", [NT1, cfg.STEP1], BF16)
    zrow = nc.dram_tensor("zrow", [128, 16], BF16)
    t2own = nc.dram_tensor("t2own", [cfg.NPAD, cfg.T2W], BF16)
    table2c = nc.dram_tensor(
        "table2c", [NBLK * cfg.NCORES * BLK, cfg.T2W], BF16,
        addr_space="Shared")
    table2 = nc.dram_tensor("table2", [NBLK * cfg.NCORES * BLK, cfg.STEP2],
                            BF16)

    first_of = [k == 0 or cb[k - 1] != cb[k] for k in range(K_pad)]
    last_of1 = [k == K1 - 1 or cb[k + 1] != cb[k] for k in range(K1)]
    last_of2 = [k == K_pad - 1 or cb[k + 1] != cb[k] for k in range(K_pad)]
    grp_of_blk = prep["grp_of_blk"]
    grp_start_blk = prep["grp_start_blk"]
    grp_base = prep["grp_base"]

    with tile.TileContext(nc) as tc:
        with ExitStack() as ctx:
            nidx_reg = nc.gpsimd.to_reg(cfg.NIDX)
            consts = ctx.enter_context(tc.tile_pool(name="consts", bufs=1))

            sb_rhs1 = consts.tile([128, cfg.W1X], BF16)
            sb_rhs2 = consts.tile([128, cfg.KT2, cfg.C2], BF16)
            sb_si1 = consts.tile([128, K_pad * 8], I16)
            sb_si2 = consts.tile([128, K_pad * 8], I16)
            # scatter one-hot slab, resident for both layers (halves loaded
            # on separate queues)
            sb_st = consts.tile([128, K_pad, 128], FP8)
            kh = K_pad // 2
            nc.sync.dma_start(
                out=sb_st[:, 0:kh, :],
                in_=st_in[:, 0:kh * 128].rearrange("p (k e) -> p k e", k=kh))
            nc.scalar.dma_start(
                out=sb_st[:, kh:K_pad, :],
                in_=st_in[:, kh * 128:].rearrange(
                    "p (k e) -> p k e", k=K_pad - kh))
            sb_ident = consts.tile([128, 128], BF16)
            sb_elown = consts.tile([128, NBLK, H1], BF16)
            sb_erown = consts.tile([128, NBLK, H1], BF16)
            sb_er2own = consts.tile([128, NBLK, 1], BF16)
            sb_er2e = consts.tile([128, K_pad], BF16)
            nc.vector.memset(sb_er2e, 0.0)

            nc.sync.dma_start(out=sb_rhs1, in_=rhs1_in[:, :])
            nc.sync.dma_start(
                out=sb_rhs2,
                in_=rhs2_in[:, :].rearrange("p (t c) -> p t c", t=cfg.KT2))
            nc.sync.dma_start(out=sb_si1, in_=si1[:, :])
            nc.sync.dma_start(out=sb_si2, in_=si2[:, :])
            make_identity(nc, sb_ident)

            # ---- stage 1: node pass -> table1 (strided 272-col writes) ----
            NSUB = cfg.NSUB
            n_super = math.ceil(cfg.NT / NSUB)
            with tc.tile_pool(name="ps_node", bufs=8, space="PSUM") as ps_node, \
                 tc.tile_pool(name="xt", bufs=4) as xtp, \
                 tc.tile_pool(name="row1", bufs=4) as rowp:
                for s in range(n_super):
                    t0 = s * NSUB * 128
                    ncols = min(NSUB * 128, N - t0)
                    nsub = math.ceil(ncols / 128)
                    xt_t = xtp.tile([128, NSUB * 128], BF16)
                    nc.sync.dma_start(out=xt_t[:, :ncols],
                                      in_=xT[:, t0:t0 + ncols])
                    # pad cols 272..384 of table1 are zero-filled by the
                    # stride-0 D2D broadcast; rows here are compact 272
                    row_b = rowp.tile([128, NSUB, cfg.W1X], BF16)
                    for i in range(nsub):
                        nt = min(128, ncols - i * 128)
                        ps = ps_node.tile([128, cfg.W1X], F32)
                        nc.tensor.matmul(ps[:nt, :],
                                         xt_t[:, i * 128:i * 128 + nt],
                                         sb_rhs1, start=True, stop=True)
                        if i % 2 == 0:
                            nc.vector.tensor_copy(row_b[:nt, i, :], ps[:nt, :])
                        else:
                            nc.scalar.copy(row_b[:nt, i, :], ps[:nt, :])
                    full = ncols // 128
                    if full:
                        dst_ap = bass.AP(
                            tensor=table1, offset=t0 * cfg.STEP1,
                            ap=[[cfg.STEP1, 128], [128 * cfg.STEP1, full],
                                [1, cfg.W1X]])
                        nc.gpsimd.dma_start(
                            out=dst_ap, in_=row_b[:, :full, :])
                    if full < nsub:
                        nt = ncols - full * 128
                        dst_ap = bass.AP(
                            tensor=table1, offset=(t0 + full * 128) * cfg.STEP1,
                            ap=[[cfg.STEP1, nt], [1, cfg.W1X]])
                        nc.gpsimd.dma_start(
                            out=dst_ap, in_=row_b[:nt, full, :])

            # zero-fill the table1 tail (rows N..NT1) so the own-slab gather
            # of the last core never reads uninitialized DRAM
            ztail = consts.tile([128, 2, cfg.STEP1], BF16)
            nc.vector.memset(ztail, 0.0)
            tail_ap = bass.AP(
                tensor=table1, offset=N * cfg.STEP1,
                ap=[[cfg.STEP1, 128], [128 * cfg.STEP1, 2], [1, cfg.STEP1]])
            nc.sync.dma_start(out=tail_ap, in_=ztail)

            # zero DRAM row: source for stride-0 broadcast zero-fills
            zrow_t = consts.tile([128, 16], BF16)
            nc.vector.memset(zrow_t, 0.0)
            nc.sync.dma_start(out=zrow[:, :], in_=zrow_t)
            # table1 pad cols (272..384) zero-filled via one D2D broadcast
    # (in first dim stride 0 re-reads the same zero row)
            half1 = NT1 // 2
            for off, cnt in ((0, half1), (half1, NT1 - half1)):
                pad_in = bass.AP(tensor=zrow, offset=0,
                                 ap=[[0, cnt], [1, cfg.STEP1 - cfg.W1X]])
                pad_out = bass.AP(tensor=table1,
                                  offset=off * cfg.STEP1 + cfg.W1X,
                                  ap=[[cfg.STEP1, cnt],
                                      [1, cfg.STEP1 - cfg.W1X]])
                nc.sync.dma_start(out=pad_out, in_=pad_in)

            # own-node el/er slab via a per-core row gather (indices
            # core*NB + i, supplied per core so the SPMD program is shared)
            ownidx = din("ownidx", [128, cfg.NPAD // 16], I16)
            sb_ownidx = consts.tile([128, cfg.NPAD // 16], I16)
            nc.sync.dma_start(out=sb_ownidx, in_=ownidx[:, :])
            nown_reg = nc.gpsimd.to_reg(cfg.NPAD)
            with tc.tile_pool(name="own", bufs=1) as own_p:
                own4 = own_p.tile([128, cfg.NPAD // 128, 64], F32)
                nc.gpsimd.dma_gather(
                    own4, table1[:, cfg.EL1:cfg.EL1 + 128].bitcast(F32),
                    sb_ownidx, cfg.NPAD, nown_reg, 64,
                    elem_step=cfg.STEP1 // 2, single_packet=False)
                own_t = own4.bitcast(BF16)
                nc.vector.tensor_copy(sb_elown, own_t[:, :, 0:H1])
                nc.vector.tensor_copy(sb_erown, own_t[:, :, H1:2 * H1])

            # ---- edge pass pools ---------------------------------------
            gt_p = ctx.enter_context(tc.tile_pool(name="gt", bufs=4))
            # sve tiles are referenced by deferred er2 matmuls up to ~4
            # batches back (last block + padding) — keep 5 bufs
            sve_p = ctx.enter_context(tc.tile_pool(name="sveb", bufs=5))
            e_p = ctx.enter_context(tc.tile_pool(name="eb", bufs=2))
            w_p = ctx.enter_context(tc.tile_pool(name="wb", bufs=3))
            fin_p = ctx.enter_context(tc.tile_pool(name="fin", bufs=2))

            # ---- stage 2: layer-1 edge pass ----------------------------
            l1ctx = ExitStack()
            ps_o = l1ctx.enter_context(tc.tile_pool(name="ps_o", bufs=2, space="PSUM"))
            ps_er = l1ctx.enter_context(tc.tile_pool(name="ps_er", bufs=2, space="PSUM"))
            ps_er2 = l1ctx.enter_context(tc.tile_pool(name="ps_er2", bufs=1, space="PSUM"))
            ps_t = l1ctx.enter_context(tc.tile_pool(name="ps_t", bufs=1, space="PSUM"))
            ps_b = l1ctx.enter_context(tc.tile_pool(name="ps_b", bufs=2, space="PSUM"))
            sve_tiles = {}
            gt = st_t = w_b = ex_b = None
            ps_acc = None
            for k in range(K1):
                g, c = divmod(k, GB)
                if c == 0:
                    gt4 = gt_p.tile([128, GB, cfg.STEP1 // 2], F32)
                    nc.gpsimd.dma_gather(
                        gt4, table1[:, :].bitcast(F32),
                        sb_si1[:, g * 128:(g + 1) * 128],
                        cfg.NIDX, nidx_reg, cfg.STEP1 // 2,
                        single_packet=False)
                    gt = gt4.bitcast(BF16)
                    sve_t = sve_p.tile([128, GB, 128], FP8, tag="sveb")
                    nc.sync.dma_start(
                        out=sve_t, in_=sve_in[:, g * 2048:(g + 1) * 2048]
                        .rearrange("p (g e) -> p g e", g=GB))
                    sve_tiles[g] = sve_t
                    # er[dst] per edge via one matmul per chunk
                    er_ps = ps_er.tile([128, GB, H1], F32)
                    for cc in range(GB):
                        bb = int(cb[g * GB + cc])
                        nc.tensor.matmul(er_ps[:, cc, :], sve_t[:, cc, :],
                                         sb_erown[:, bb, :],
                                         start=True, stop=True)
                    # e = lrelu(el + er);  ex = exp(e)
                    e_b = e_p.tile([128, GB, H1], F32, tag="e1")
                    nc.vector.tensor_add(
                        e_b, gt[:, :, cfg.EL1:cfg.EL1 + H1], er_ps)
                    e_l = e_p.tile([128, GB, H1], F32, tag="e1")
                    nc.vector.scalar_tensor_tensor(
                        e_l, e_b, cfg.NEG, e_b, ALU.mult, ALU.max)
                    ex_b = e_p.tile([128, GB, H1], BF16, tag="e1")
                    nc.scalar.activation(ex_b, e_l, ACTF.Exp)
                    # w rows: [ft*ex (d-major) | ex]
                    w_b = w_p.tile([128, GB, FT1 + H1], BF16, tag="wb")
                    nc.vector.tensor_mul(
                        w_b[:, :, 0:FT1].rearrange("p g (d h) -> p g d h", d=D1),
                        gt[:, :, 0:FT1].rearrange("p g (d h) -> p g d h", d=D1),
                        _bcast(ex_b, 2, D1))
                    nc.scalar.copy(w_b[:, :, FT1:FT1 + H1], ex_b)

                if k == K1 - 1:
                    # pre-zero table2 cols (relayout writes 21 of 128; the
                    # layer-2 gather reads whole 256B rows)
                    t2rows = NBLK * cfg.NCORES * BLK
                    for zoff in (0, t2rows // 2):
                        z_in = bass.AP(tensor=zrow, offset=0,
                                       ap=[[0, t2rows // 2], [1, cfg.STEP2]])
                        z_out = bass.AP(tensor=table2,
                                        offset=zoff * cfg.STEP2,
                                        ap=[[cfg.STEP2, t2rows // 2],
                                            [1, cfg.STEP2]])
                        nc.gpsimd.dma_start(out=z_out, in_=z_in)

                b = int(cb[k])
                if first_of[k]:
                    ps_acc = ps_o.tile([128, FT1 + H1], F32)
                nc.tensor.matmul(ps_acc, sb_st[:, k, :], w_b[:, c, :],
                                 start=first_of[k], stop=last_of1[k])
                if last_of1[k]:
                    # h = relu(num / max(den, eps))
                    s_t = fin_p.tile([128, H1], F32, tag="fin")
                    nc.vector.tensor_scalar_max(
                        s_t, ps_acc[:, FT1:FT1 + H1], 1e-16)
                    rs = fin_p.tile([128, H1], F32, tag="fin")
                    nc.vector.reciprocal(rs, s_t)
                    ht = fin_p.tile([128, FT1], F32, tag="fin")
                    nc.vector.tensor_mul(
                        ht.rearrange("p (d h) -> p d h", d=D1),
                        ps_acc[:, 0:FT1].rearrange("p (d h) -> p d h", d=D1),
                        _bcast(rs, 1, D1))
                    sb_h = fin_p.tile([128, FT1], BF16, tag="hblk")
                    nc.scalar.activation(sb_h, ht, ACTF.Relu)

                    # layer-2 node prep for block b
                    sb_hT = fin_p.tile([128, cfg.KT2, 128], BF16, tag="hTblk")
                    for half in range(cfg.KT2):
                        pst = ps_t.tile([128, 128], BF16)
                        nc.tensor.transpose(
                            pst, sb_h[:, half * 128:(half + 1) * 128],
                            sb_ident)
                        nc.scalar.copy(sb_hT[:, half, :], pst)
                    ps2 = ps_b.tile([128, cfg.C2], F32)
                    for half in range(cfg.KT2):
                        nc.tensor.matmul(
                            ps2, sb_hT[:, half, :], sb_rhs2[:, half, :],
                            start=(half == 0), stop=(half == cfg.KT2 - 1))
                    row2 = fin_p.tile([128, cfg.T2W], BF16, tag="fin2")
                    nc.scalar.copy(row2[:, 0:C // 2].bitcast(FP8), ps2[:, 0:C])
                    nc.scalar.copy(row2[:, C // 2:C // 2 + 1], ps2[:, C:C + 1])
                    nc.scalar.copy(sb_er2own[:, b, :], ps2[:, C + 1:C + 2])
                    nc.sync.dma_start(
                        out=t2own[b * BLK:(b + 1) * BLK, 0:cfg.T2W], in_=row2)

                    # deferred er2 distribution for block b's chunks
                    ch_l = prep["blk_chunks"][b]
                    er2_ps = ps_er2.tile([128, max(16, len(ch_l))], F32)
                    for j, kk in enumerate(ch_l):
                        gg, cc = divmod(kk, GB)
                        nc.tensor.matmul(
                            er2_ps[:, j:j + 1], sve_tiles[gg][:, cc, :],
                            sb_er2own[:, b, :], start=True, stop=True)
                    k0 = ch_l[0]
                    nc.vector.tensor_copy(
                        sb_er2e[:, k0:k0 + len(ch_l)],
                        er2_ps[:, 0:len(ch_l)])

                    # grouped allgather once a group's blocks are all done
                    gidx = int(grp_of_blk[b])
                    if b == int(grp_start_blk[gidx]) + cfg.GRPS[gidx] - 1:
                        b0 = int(grp_start_blk[gidx])
                        nblks = cfg.GRPS[gidx]
                        rows = nblks * BLK
                        base = int(grp_base[gidx])
                        nc.gpsimd.collective_compute(
                            "AllGather", ALU.bypass,
                            replica_groups=[list(range(cfg.NCORES))],
                            ins=[t2own[b0 * BLK:b0 * BLK + rows, :].opt()],
                            outs=[table2c[base:base + cfg.NCORES * rows, :].opt()])
                        # relayout compact 42-byte rows -> 256B-stride rows
                        # (NEFF requires contiguous collective outputs)
                        nrows = cfg.NCORES * rows
                        src_ap = bass.AP(
                            tensor=table2c, offset=base * cfg.T2W,
                            ap=[[cfg.T2W, nrows], [1, cfg.T2W]])
                        dst_ap = bass.AP(
                            tensor=table2, offset=base * cfg.STEP2,
                            ap=[[cfg.STEP2, nrows], [1, cfg.T2W]])
                        nc.sync.dma_start(out=dst_ap, in_=src_ap)

            # ---- stage 3: layer-2 edge pass ----------------------------
            l1ctx.close()
            ps_o2 = ctx.enter_context(
                tc.tile_pool(name="ps_o2", bufs=2, space="PSUM"))
            GB2 = cfg.GB2
            nidx2_reg = nc.gpsimd.to_reg(cfg.NIDX2)
            g2 = w2_b = None
            ps_acc2 = None
            HB = 16                              # compute half-batch
            for k in range(K_pad):
                g, c = divmod(k, GB2)
                if c == 0:
                    g24 = gt_p.tile([128, GB2, cfg.STEP2 // 2], F32, tag="gt")
                    nc.gpsimd.dma_gather(
                        g24, table2[:, :].bitcast(F32),
                        sb_si2[:, g * 128:(g + 1) * 128],
                        cfg.NIDX2, nidx2_reg, cfg.STEP2 // 2,
                        single_packet=False)
                    g2 = g24.bitcast(BF16)
                    w2_b = w_p.tile([128, GB2, C + 1], BF16, tag="wb2")
                if c % HB == 0:
                    h0 = c
                    g2h = g2[:, h0:h0 + HB, :]
                    e2 = e_p.tile([128, HB, 1], F32, tag="e2")
                    nc.vector.tensor_add(
                        e2, g2h[:, :, C // 2:C // 2 + 1],
                        _bcast(sb_er2e[:, g * GB2 + h0:g * GB2 + h0 + HB], 2, 1))
                    e2l = e_p.tile([128, HB, 1], F32, tag="e2")
                    nc.vector.scalar_tensor_tensor(
                        e2l, e2, cfg.NEG, e2, ALU.mult, ALU.max)
                    ex2 = e_p.tile([128, HB, 1], BF16, tag="e2")
                    nc.scalar.activation(ex2, e2l, ACTF.Exp)
                    meng = nc.vector if (g % 2 == 0) else nc.gpsimd
                    meng.tensor_mul(
                        w2_b[:, h0:h0 + HB, 0:C],
                        g2h[:, :, 0:C // 2].bitcast(FP8),
                        _bcast(ex2.squeeze(2), 2, C))
                    nc.vector.tensor_copy(
                        w2_b[:, h0:h0 + HB, C:C + 1], ex2)

                b = int(cb[k])
                if first_of[k]:
                    ps_acc2 = ps_o2.tile([128, C + 1], F32)
                nc.tensor.matmul(ps_acc2, sb_st[:, k, :], w2_b[:, c, :],
                                 start=first_of[k], stop=last_of2[k])
                if last_of2[k]:
                    s2 = fin_p.tile([128, 1], F32, tag="finL2")
                    nc.vector.tensor_scalar_max(s2, ps_acc2[:, C:C + 1], 1e-16)
                    rs2 = fin_p.tile([128, 1], F32, tag="finL2")
                    nc.vector.reciprocal(rs2, s2)
                    ot = fin_p.tile([128, C], F32, tag="finL2")
                    nc.vector.tensor_mul(ot, ps_acc2[:, 0:C],
                                         _bcast(rs2.squeeze(1), 1, C))
                    nc.sync.dma_start(out=out[b * BLK:(b + 1) * BLK, :], in_=ot)

    nc.finalize()
    return nc


# ---------------------------------------------------------------- driver

def _make_in_maps(cfg, inputs, prep):
    x = np.asarray(inputs["x"], np.float32)
    W1 = np.asarray(inputs["W1"], np.float32)
    al1 = np.asarray(inputs["attn_l1"], np.float32)
    ar1 = np.asarray(inputs["attn_r1"], np.float32)
    W2 = np.asarray(inputs["W2"], np.float32)
    al2 = np.asarray(inputs["attn_l2"], np.float32).reshape(-1)
    ar2 = np.asarray(inputs["attn_r2"], np.float32).reshape(-1)

    pm = _dmajor_perm(cfg.H1, cfg.D1)
    W1p = W1[:, pm]                                    # d-major ft cols
    el_cols = np.einsum("fhd,hd->fh", W1.reshape(cfg.F, cfg.H1, cfg.D1), al1)
    er_cols = np.einsum("fhd,hd->fh", W1.reshape(cfg.F, cfg.H1, cfg.D1), ar1)
    rhs1 = np.concatenate([W1p, el_cols, er_cols], axis=1)

    W2p = W2[pm, :]                                    # K rows d-major
    el2_col = W2 @ al2                                 # [256]
    er2_col = W2 @ ar2
    rhs2 = np.concatenate(
        [W2p, el2_col[pm, None], er2_col[pm, None]], axis=1)  # [256, 42]
    rhs2 = np.ascontiguousarray(
        rhs2.reshape(cfg.KT2, 128, cfg.C2).transpose(1, 0, 2).reshape(
            128, cfg.KT2 * cfg.C2))

    common = {
        "xT": np.ascontiguousarray(x.T).astype(NP_BF16),
        "rhs1": np.ascontiguousarray(rhs1).astype(NP_BF16),
        "rhs2": rhs2.astype(NP_BF16),
    }
    in_maps = []
    for c in range(cfg.NCORES):
        m = dict(common)
        m["st"] = prep["st"][c]
        m["sve"] = prep["sve"][c]
        m["si1"] = prep["si1"][c]
        m["si2"] = prep["si2"][c]
        m["ownidx"] = _wrap16(np.arange(c * cfg.NB, c * cfg.NB + cfg.NPAD))
        in_maps.append(m)
    return in_maps


def build_all(inputs, cfg=CFG):
    prep = _host_prep(cfg, inputs["src"], inputs["dst"])
    nc = _build(cfg, prep)
    in_maps = _make_in_maps(cfg, inputs, prep)
    return nc, in_maps


def kernel(**inputs):
    cfg = CFG
    nc, in_maps = build_all(inputs, cfg)
    from concourse.bass_utils import run_bass_kernel_spmd
    res = run_bass_kernel_spmd(nc, in_maps, core_ids=list(range(cfg.NCORES)))
    out = np.concatenate(
        [res.results[c]["out"][:cfg.NB] for c in range(cfg.NCORES)], axis=0)
    return np.ascontiguousarray(out, dtype=np.float32)
